# revision 29
# baseline (speedup 1.0000x reference)
"""HGAT layer (hyperbolic GAT) kernel for trn2, 8 NeuronCores, full-device.

Single SPMD bass/Tile program on 8 cores:
  phase 1  per core: its 6250 rows of x through HypLinear
           (logmap0 -> @W.T -> expmap0 -> proj -> mobius+hb -> proj ->
           logmap0) -> L rows + per-64-chunk dots with att (vLi/vLj).
           Column math ([128,1] scalars) is batched across the 49 row
           tiles into [128,49] pipelines.
  AllGather L|vLi|vLj.
  phase 2  strided-DMA permute into Gmat row order (the reference's
           view(heads,-1,DH) quirk): gather table T=[sj|Gmat] and
           si_full.
  phase 3  edge-parallel segment softmax. Edges sorted by dst and
           grouped into 128-slot dst blocks (padded per block). Per 256
           edges: indirect-gather T[src] rows and si_full[dst] rows,
           build one-hot(dst_local) vs an iota row, w =
           exp(leaky_relu(si+sj, 0.2)), PSUM-accumulate one-hot.T @
           [w | w*msg] -> per-node denominator and weighted sum.
  AllGather aggregates.
  phase 4  inverse permutation via indexed quarter-row gathers, +b_conv,
           relu, expmap0, proj -> output (fp16 download).

Host only sorts/pads the edge list and packs small constants; all
core-dependent addressing is index DATA uploaded per core, so one
program serves all cores.
"""
import os
import time
import numpy as np

N, E, DIN, H, DH = 50000, 800000, 256, 4, 64
MIN_NORM = 1e-15
PROJ_EPS = 4e-3
P = 128
RPC = 6250            # real rows per core
RPAD = 6272           # padded rows per core (49 tiles)
NBLK = RPAD // P      # 49 dst blocks per core
NC_ = 8
NPAD = NC_ * RPAD     # 50176
PADROW = N            # gather-table pad row index
TW = 260              # table row: [sj(4) | msg(256)]
IG = 4                # edge iters per index-DMA group

_TIME = os.environ.get("HGAT_TIME") == "1"


def _tick(tag, t0):
    if _TIME:
        print(f"  [hgat] {tag}: {time.time()-t0:.3f}s", flush=True)
    return time.time()


# ---------------- host-side hyperbolic helpers (small tensors only) -------
def _norm(a):
    return np.clip(np.linalg.norm(a, axis=-1, keepdims=True), MIN_NORM, None)


def _logmap0(a):
    n = _norm(a)
    return np.arctanh(np.minimum(n, 1 - 1e-7)) * a / n


def _expmap0(u):
    n = _norm(u)
    return np.tanh(n) * u / n


def _proj(a):
    n = _norm(a)
    mx = 1.0 - PROJ_EPS
    return np.where(n > mx, a / n * mx, a)


# ---------------- tile framework patches (walrus: 1 sem wait / inst) ------
def _install_tile_patches():
    import concourse.tile as tile
    from concourse import mybir
    from concourse.vector_clock import ScopedClock, VectorClock

    if getattr(tile.TileContext, "_hgat_patched", False):
        return

    def _drain_and_barrier_split(self, tick_clock, wait_clock):
        gc = tick_clock.global_clock
        n = len(gc)
        for i in range(n):
            t = gc[i]
            if t <= 0:
                continue
            vc = VectorClock([0] * n)
            vc.require_at_least(i, t)
            d = self.nc.sync.drain()
            wait_clock.add_sem_waits(d.ins, ScopedClock({None: vc}))
        self.nc.all_engine_barrier()
        assert self.sems is not None
        popped = self.nc._tile_sem_poison_stack.pop()
        assert popped is self._sem_poison
        self.nc.clear_and_free_semaphores(list(self.sems.allocated().values()))
        self.nc.all_engine_barrier()

    _orig_add = tile.TileContext._add_instruction

    def _add_instruction_split(self, inst):
        si = inst.sync_info
        if si is not None and len(si.on_wait) > 1:
            waits = list(si.on_wait)
            for k, w in enumerate(waits[:-1]):
                carrier = mybir.InstEventSemaphore(
                    name=f"{inst.name}-sw{k}",
                    engine=inst.engine,
                    ins=[],
                    outs=[],
                    sync_info=mybir.SyncInfo(on_wait=[w], on_update=[]),
                )
                _orig_add(self, carrier)
            inst.sync_info = mybir.SyncInfo(
                on_wait=[waits[-1]], on_update=list(si.on_update)
            )
        return _orig_add(self, inst)

    tile.TileContext._drain_and_barrier = _drain_and_barrier_split
    tile.TileContext._add_instruction = _add_instruction_split
    tile.TileContext._hgat_patched = True


# ---------------- bass program ------------------------------------------
def _build_nc(tpb2, epad, y2):
    """tpb2: per dst block, number of 256-edge iterations (same all cores)."""
    from concourse import bass, mybir
    from concourse.bass import IndirectOffsetOnAxis
    from concourse.masks import make_identity
    import concourse.tile as tile

    _install_tile_patches()

    f32 = mybir.dt.float32
    f16 = mybir.dt.float16
    i32 = mybir.dt.int32
    AF = mybir.ActivationFunctionType
    AX = mybir.AxisListType
    OP = mybir.AluOpType

    n_iters = int(sum(tpb2))
    n_groups = (n_iters + IG - 1) // IG

    nc = bass.Bass("TRN2", target_bir_lowering=False, debug=False, num_devices=NC_)
    x_in = nc.dram_tensor("x_in", [RPAD, DIN], f16, kind="ExternalInput")
    w_in = nc.dram_tensor("w_in", [DIN, DIN], f32, kind="ExternalInput")
    hb_in = nc.dram_tensor("hb_in", [P, DIN], f32, kind="ExternalInput")
    ai_in = nc.dram_tensor("ai_in", [P, DIN], f32, kind="ExternalInput")
    aj_in = nc.dram_tensor("aj_in", [P, DIN], f32, kind="ExternalInput")
    bc_in = nc.dram_tensor("bc_in", [P, DIN], f32, kind="ExternalInput")
    ed_in = nc.dram_tensor("ed_in", [P, n_groups * 6 * IG], i32, kind="ExternalInput")
    fidx_in = nc.dram_tensor("fidx_in", [P, NBLK * 4], i32, kind="ExternalInput")
    y_out = nc.dram_tensor("y_out", [RPAD, DIN], f16, kind="ExternalOutput")

    with tile.TileContext(nc) as tc:
        with (
            tc.tile_pool(name="dram", bufs=1, space="DRAM") as dram,
            tc.tile_pool(name="const", bufs=1) as cp,
        ):
            lv_loc = dram.tile([RPAD, DIN + 8], f32, tag="lv_loc", name="lv_loc")
            lv_full = dram.tile([NPAD, DIN + 8], f32, tag="lv_full", name="lv_full")
            t_tab = dram.tile([N + 1, TW], f32, tag="t_tab", name="t_tab")
            si_full = dram.tile([NPAD, 4], f32, tag="si_full", name="si_full")
            og_loc = dram.tile([RPAD, DIN], f32, tag="og_loc", name="og_loc")
            og_full = dram.tile([NPAD, DIN], f32, tag="og_full", name="og_full")

            ident = cp.tile([P, P], f32, tag="id", name="ident")
            make_identity(nc, ident[:])
            iota_i = cp.tile([P, P], i32, tag="ioi", name="iota_i")
            nc.gpsimd.iota(iota_i[:], pattern=[[1, P]], base=0, channel_multiplier=0)
            iota_row = cp.tile([P, P], f32, tag="ior", name="iota_row")
            nc.vector.tensor_copy(out=iota_row[:], in_=iota_i[:])
            w0 = cp.tile([P, DIN], f32, tag="w0", name="w0")
            w1 = cp.tile([P, DIN], f32, tag="w1", name="w1")
            nc.sync.dma_start(out=w0[:], in_=w_in[0:P, :])
            nc.sync.dma_start(out=w1[:], in_=w_in[P:DIN, :])
            hbt = cp.tile([P, DIN], f32, tag="hb", name="hbt")
            nc.sync.dma_start(out=hbt[:], in_=hb_in[:])
            ait = cp.tile([P, DIN], f32, tag="ai", name="ait")
            nc.sync.dma_start(out=ait[:], in_=ai_in[:])
            ajt = cp.tile([P, DIN], f32, tag="aj", name="ajt")
            nc.sync.dma_start(out=ajt[:], in_=aj_in[:])
            bct = cp.tile([P, DIN], f32, tag="bc", name="bct")
            nc.sync.dma_start(out=bct[:], in_=bc_in[:])
            fidxt = cp.tile([P, NBLK * 4], i32, tag="fidx", name="fidxt")
            nc.sync.dma_start(out=fidxt[:], in_=fidx_in[:])

            # ======== phase 1: dense (column math batched over tiles) =====
            with (
                tc.tile_pool(name="p1y", bufs=NBLK) as ypool,
                tc.tile_pool(name="p1h", bufs=NBLK) as hpool,
                tc.tile_pool(name="p1sb", bufs=3) as sb,
                tc.tile_pool(name="p1c", bufs=2) as colp,
                tc.tile_pool(name="p1ps", bufs=2, space="PSUM") as ps,
                tc.tile_pool(name="p1py", bufs=2, space="PSUM") as py,
            ):
                def cols(tag):
                    return colp.tile([P, NBLK], f32, tag=tag, name=tag)

                n2c = cols("n2c")
                for t in range(NBLK):
                    xt = sb.tile([P, DIN], f16, tag="x", name="xt")
                    nc.sync.dma_start(out=xt[:], in_=x_in[t * P:(t + 1) * P, :])
                    sq = sb.tile([P, DIN], f32, tag="sq", name="sq")
                    nc.scalar.activation(out=sq[:], in_=xt[:], func=AF.Square,
                                         accum_out=n2c[:, t:t + 1])

                # pipeline A: s = atanh(min(max(sqrt(n2),MN),1-eps)) / n
                nA = cols("nA")
                nc.scalar.activation(out=nA[:], in_=n2c[:], func=AF.Sqrt)
                nc.vector.tensor_scalar_max(out=nA[:], in0=nA[:], scalar1=MIN_NORM)
                nmA = cols("nmA")
                nc.vector.tensor_scalar_min(out=nmA[:], in0=nA[:], scalar1=1 - 1e-7)
                a1A = cols("a1A")
                nc.vector.tensor_scalar_add(out=a1A[:], in0=nmA[:], scalar1=1.0)
                a2A = cols("a2A")
                nc.vector.tensor_scalar(out=a2A[:], in0=nmA[:], scalar1=-1.0,
                                        scalar2=1.0, op0=OP.mult, op1=OP.add)
                nc.vector.reciprocal(out=a2A[:], in_=a2A[:])
                nc.vector.tensor_tensor(out=a1A[:], in0=a1A[:], in1=a2A[:], op=OP.mult)
                nc.scalar.activation(out=a1A[:], in_=a1A[:], func=AF.Ln)
                nc.vector.reciprocal(out=nA[:], in_=nA[:])
                sA = cols("sA")
                nc.vector.tensor_tensor(out=sA[:], in0=a1A[:], in1=nA[:], op=OP.mult)
                nc.vector.tensor_scalar_mul(out=sA[:], in0=sA[:], scalar1=0.5)

                # pass 2: matmul + expmap0 norm (x re-streamed from DRAM)
                n2y = cols("n2y")
                yts = []
                for t in range(NBLK):
                    xt = sb.tile([P, DIN], f16, tag="x", name="xt")
                    nc.sync.dma_start(out=xt[:], in_=x_in[t * P:(t + 1) * P, :])
                    lx = sb.tile([P, DIN], f32, tag="lx", name="lx")
                    nc.vector.tensor_scalar_mul(out=lx[:], in0=xt[:],
                                                scalar1=sA[:, t:t + 1])
                    yp = py.tile([P, DIN], f32, tag="yp", name="yp")
                    for c in range(2):
                        tp = ps.tile([P, P], f32, tag="tp", name="tp")
                        nc.tensor.transpose(out=tp[:], in_=lx[:, c * P:(c + 1) * P],
                                            identity=ident[:])
                        lxT = sb.tile([P, P], f32, tag="lxT", name="lxT")
                        nc.scalar.copy(out=lxT[:], in_=tp[:])
                        nc.tensor.matmul(out=yp[:], lhsT=lxT[:],
                                         rhs=(w0 if c == 0 else w1)[:],
                                         start=(c == 0), stop=(c == 1))
                    sq2 = sb.tile([P, DIN], f32, tag="sq", name="sq2")
                    nc.scalar.activation(out=sq2[:], in_=yp[:], func=AF.Square,
                                         accum_out=n2y[:, t:t + 1])
                    yt = ypool.tile([P, DIN], f32, tag="y", name="yt")
                    nc.vector.tensor_copy(out=yt[:], in_=yp[:])
                    yts.append(yt)

                # pipeline B: s2 = min(tanh(ny),1-eps)/ny ; x2 = min(..)^2
                nyB = cols("nyB")
                nc.scalar.activation(out=nyB[:], in_=n2y[:], func=AF.Sqrt)
                nc.vector.tensor_scalar_max(out=nyB[:], in0=nyB[:], scalar1=MIN_NORM)
                thB = cols("thB")
                nc.scalar.activation(out=thB[:], in_=nyB[:], func=AF.Tanh)
                nc.vector.tensor_scalar_min(out=thB[:], in0=thB[:],
                                            scalar1=1 - PROJ_EPS)
                nc.vector.reciprocal(out=nyB[:], in_=nyB[:])
                s2B = cols("s2B")
                nc.vector.tensor_tensor(out=s2B[:], in0=thB[:], in1=nyB[:], op=OP.mult)
                x2B = cols("x2B")
                nc.vector.tensor_tensor(out=x2B[:], in0=thB[:], in1=thB[:], op=OP.mult)

                # pass 3: mobius dot products
                xyC = cols("xyC")
                hts = []
                for t in range(NBLK):
                    xh1 = hpool.tile([P, DIN], f32, tag="xh1", name="xh1")
                    nc.vector.tensor_scalar_mul(out=xh1[:], in0=yts[t][:],
                                                scalar1=s2B[:, t:t + 1])
                    mm = sb.tile([P, DIN], f32, tag="mm", name="mm")
                    nc.vector.tensor_tensor(out=mm[:], in0=xh1[:], in1=hbt[:],
                                            op=OP.mult)
                    nc.vector.reduce_sum(out=xyC[:, t:t + 1], in_=mm[:], axis=AX.X)
                    hts.append(xh1)

                # pipeline C: mobius scalar chain
                a1m = cols("a1m")
                nc.vector.tensor_scalar(out=a1m[:], in0=xyC[:], scalar1=2.0,
                                        scalar2=1.0 + y2, op0=OP.mult, op1=OP.add)
                a2m = cols("a2m")
                nc.vector.tensor_scalar(out=a2m[:], in0=x2B[:], scalar1=-1.0,
                                        scalar2=1.0, op0=OP.mult, op1=OP.add)
                d1m = cols("d1m")
                nc.vector.tensor_scalar(out=d1m[:], in0=x2B[:], scalar1=y2,
                                        scalar2=1.0, op0=OP.mult, op1=OP.add)
                denm = cols("denm")
                nc.vector.tensor_scalar(out=denm[:], in0=xyC[:], scalar1=2.0,
                                        scalar2=0.0, op0=OP.mult, op1=OP.add)
                nc.vector.tensor_tensor(out=denm[:], in0=denm[:], in1=d1m[:], op=OP.add)
                nc.vector.reciprocal(out=denm[:], in_=denm[:])

                # pass 4: xh2 = (a1m*xh1 + a2m*hb) * rden ; its norm
                n2x = cols("n2x")
                for t in range(NBLK):
                    xh1 = hts[t]
                    t1 = sb.tile([P, DIN], f32, tag="t1", name="t1")
                    nc.vector.tensor_scalar_mul(out=t1[:], in0=xh1[:],
                                                scalar1=a1m[:, t:t + 1])
                    t2 = sb.tile([P, DIN], f32, tag="t2", name="t2")
                    nc.vector.tensor_scalar_mul(out=t2[:], in0=hbt[:],
                                                scalar1=a2m[:, t:t + 1])
                    nc.vector.tensor_tensor(out=t1[:], in0=t1[:], in1=t2[:], op=OP.add)
                    # reuse xh1 slot for xh2
                    nc.vector.tensor_scalar_mul(out=xh1[:], in0=t1[:],
                                                scalar1=denm[:, t:t + 1])
                    sq3 = sb.tile([P, DIN], f32, tag="sq3", name="sq3")
                    nc.scalar.activation(out=sq3[:], in_=xh1[:], func=AF.Square,
                                         accum_out=n2x[:, t:t + 1])

                # pipeline D: s3 = atanh(min(max(sqrt,MN),1-PROJ_EPS)) / n
                nxD = cols("nxD")
                nc.scalar.activation(out=nxD[:], in_=n2x[:], func=AF.Sqrt)
                nc.vector.tensor_scalar_max(out=nxD[:], in0=nxD[:], scalar1=MIN_NORM)
                ncD = cols("ncD")
                nc.vector.tensor_scalar_min(out=ncD[:], in0=nxD[:],
                                            scalar1=1 - PROJ_EPS)
                b1D = cols("b1D")
                nc.vector.tensor_scalar_add(out=b1D[:], in0=ncD[:], scalar1=1.0)
                b2D = cols("b2D")
                nc.vector.tensor_scalar(out=b2D[:], in0=ncD[:], scalar1=-1.0,
                                        scalar2=1.0, op0=OP.mult, op1=OP.add)
                nc.vector.reciprocal(out=b2D[:], in_=b2D[:])
                nc.vector.tensor_tensor(out=b1D[:], in0=b1D[:], in1=b2D[:], op=OP.mult)
                nc.scalar.activation(out=b1D[:], in_=b1D[:], func=AF.Ln)
                nc.vector.reciprocal(out=nxD[:], in_=nxD[:])
                s3D = cols("s3D")
                nc.vector.tensor_tensor(out=s3D[:], in0=b1D[:], in1=nxD[:], op=OP.mult)
                nc.vector.tensor_scalar_mul(out=s3D[:], in0=s3D[:], scalar1=0.5)

                # pass 5: L, vLi, vLj, store
                for t in range(NBLK):
                    lv = sb.tile([P, DIN + 8], f32, tag="lv", name="lv")
                    nc.vector.tensor_scalar_mul(out=lv[:, 0:DIN], in0=hts[t][:],
                                                scalar1=s3D[:, t:t + 1])
                    mi = sb.tile([P, DIN], f32, tag="mi", name="mi")
                    nc.vector.tensor_tensor(out=mi[:], in0=lv[:, 0:DIN], in1=ait[:],
                                            op=OP.mult)
                    mj = sb.tile([P, DIN], f32, tag="mj", name="mj")
                    nc.vector.tensor_tensor(out=mj[:], in0=lv[:, 0:DIN], in1=ajt[:],
                                            op=OP.mult)
                    nc.vector.reduce_sum(
                        out=lv[:, DIN:DIN + 4],
                        in_=mi[:].rearrange("p (h d) -> p h d", h=4), axis=AX.X)
                    nc.vector.reduce_sum(
                        out=lv[:, DIN + 4:DIN + 8],
                        in_=mj[:].rearrange("p (h d) -> p h d", h=4), axis=AX.X)
                    nc.sync.dma_start(out=lv_loc[t * P:(t + 1) * P, :], in_=lv[:])

            # ======== AllGather L|vLi|vLj =================================
            nc.gpsimd.collective_compute(
                "AllGather", mybir.AluOpType.bypass,
                replica_groups=[list(range(NC_))],
                ins=[lv_loc.opt()], outs=[lv_full.opt()])

            # ======== phase 2: permutations ===============================
            # T[i, 4 + h*64 + d] = L[node h*12500 + i//4, (i%4)*64 + d]
            for h in range(H):
                for q in range(2):
                    r0 = (2 * h + q) * RPAD
                    src = lv_full[r0:r0 + RPC, 0:DIN].rearrange(
                        "a (b d) -> a b d", b=4)
                    dst = t_tab[q * 25000:(q + 1) * 25000,
                                4 + h * DH:4 + (h + 1) * DH
                                ].rearrange("(a b) d -> a b d", b=4)
                    nc.sync.dma_start(out=dst, in_=src)
                    srcj = lv_full[r0:r0 + RPC, DIN + 4:DIN + 8]
                    dstj = t_tab[q * 25000:(q + 1) * 25000, h:h + 1
                                 ].rearrange("(a b) one -> a (b one)", b=4)
                    nc.sync.dma_start(out=dstj, in_=srcj)
                    srci = lv_full[r0:r0 + RPC, DIN:DIN + 4]
                    dsti = si_full[q * 25000:(q + 1) * 25000, h:h + 1
                                   ].rearrange("(a b) one -> a (b one)", b=4)
                    nc.sync.dma_start(out=dsti, in_=srci)
            padt = cp.tile([P, TW], f32, tag="padt", name="padt")
            nc.vector.memset(padt[:], 0.0)
            nc.vector.memset(padt[0:1, 0:4], -1000.0)
            nc.sync.dma_start(out=t_tab[N:N + 1, :], in_=padt[0:1, :])

            # ======== phase 3: edge loop ==================================
            with (
                tc.tile_pool(name="esb", bufs=4) as eb,
                tc.tile_pool(name="eid", bufs=1) as eip,
                tc.tile_pool(name="eac", bufs=2, space="PSUM") as eac,
            ):
                idxg = eip.tile([P, n_groups * 6 * IG], i32, tag="idxg",
                                name="idxg")
                nc.sync.dma_start(out=idxg[:], in_=ed_in[:])
                it = 0
                for b0 in range(NBLK):
                    niter = tpb2[b0]
                    acc = eac.tile([P, TW], f32, tag="acc", name="acc")
                    for k in range(niter):
                        # per-iter cols: [src_lo, src_hi, dstg_lo,
                        #                 dstg_hi, dlf_lo, dlf_hi]
                        c0 = it * 6
                        g2 = eb.tile([P, 2 * TW], f32, tag="g2", name="g2")
                        sie = eb.tile([P, 8], f32, tag="sie", name="sie")
                        for half in range(2):
                            nc.gpsimd.indirect_dma_start(
                                out=g2[:, half * TW:(half + 1) * TW],
                                out_offset=None, in_=t_tab[:],
                                in_offset=IndirectOffsetOnAxis(
                                    ap=idxg[:, c0 + half:c0 + half + 1],
                                    axis=0))
                            nc.gpsimd.indirect_dma_start(
                                out=sie[:, half * 4:(half + 1) * 4],
                                out_offset=None, in_=si_full[:],
                                in_offset=IndirectOffsetOnAxis(
                                    ap=idxg[:, c0 + 2 + half:c0 + 3 + half],
                                    axis=0))
                        dstf = idxg[:, c0 + 4:c0 + 6].bitcast(f32)
                        oh = eb.tile([P, 2 * P], f32, tag="oh", name="oh")
                        nc.vector.tensor_tensor(
                            out=oh[:].rearrange("p (t q) -> p t q", t=2),
                            in0=dstf[:, :, None].to_broadcast([P, 2, P]),
                            in1=iota_row[:, None, :].to_broadcast([P, 2, P]),
                            op=OP.is_equal)
                        nc.vector.tensor_tensor(
                            out=sie[:].rearrange("p (t c) -> p t c", t=2),
                            in0=sie[:].rearrange("p (t c) -> p t c", t=2),
                            in1=g2[:].rearrange("p (t c) -> p t c", t=2)[:, :, 0:4],
                            op=OP.add)
                        nc.scalar.activation(out=sie[:], in_=sie[:],
                                             func=AF.Prelu, alpha=0.2)
                        rhs = eb.tile([P, 2 * TW], f32, tag="rhs", name="rhs")
                        nc.scalar.activation(
                            out=rhs[:].rearrange("p (t c) -> p t c", t=2)[:, :, 0:4],
                            in_=sie[:].rearrange("p (t c) -> p t c", t=2),
                            func=AF.Exp)
                        nc.vector.tensor_tensor(
                            out=rhs[:].rearrange("p (t c) -> p t c", t=2)[:, :, 4:TW
                                ].rearrange("p t (h d) -> p t h d", h=4),
                            in0=g2[:].rearrange("p (t c) -> p t c", t=2)[:, :, 4:TW
                                ].rearrange("p t (h d) -> p t h d", h=4),
                            in1=rhs[:].rearrange("p (t c) -> p t c", t=2)[:, :, 0:4
                                ][:, :, :, None].to_broadcast([P, 2, 4, DH]),
                            op=OP.mult)
                        for half in range(2):
                            nc.tensor.matmul(
                                out=acc[:],
                                lhsT=oh[:, half * P:(half + 1) * P],
                                rhs=rhs[:, half * TW:(half + 1) * TW],
                                start=(k == 0 and half == 0),
                                stop=(k == niter - 1 and half == 1))
                        it += 1
                    rden4 = eb.tile([P, 4], f32, tag="rden4", name="rden4")
                    nc.vector.reciprocal(out=rden4[:], in_=acc[:, 0:4])
                    og = eb.tile([P, DIN], f32, tag="og", name="og")
                    nc.vector.tensor_tensor(
                        out=og[:].rearrange("p (h d) -> p h d", h=4),
                        in0=acc[:, 4:TW].rearrange("p (h d) -> p h d", h=4),
                        in1=rden4[:, :, None].to_broadcast([P, 4, DH]), op=OP.mult)
                    nc.sync.dma_start(out=og_loc[b0 * P:(b0 + 1) * P, :], in_=og[:])

            # ======== AllGather aggregates ================================
            nc.gpsimd.collective_compute(
                "AllGather", mybir.AluOpType.bypass,
                replica_groups=[list(range(NC_))],
                ins=[og_loc.opt()], outs=[og_full.opt()])

            # ======== phase 4: inverse permutation + final ================
            with (
                tc.tile_pool(name="fsb", bufs=3) as fb,
                tc.tile_pool(name="ffp", bufs=NBLK) as fpp,
                tc.tile_pool(name="fc", bufs=2) as fcolp,
            ):
                ogq = og_full[:].rearrange("n (q d) -> (n q) d", q=4)

                n2f = fcolp.tile([P, NBLK], f32, tag="n2f", name="n2f")
                fts = []
                for t in range(NBLK):
                    fp = fpp.tile([P, DIN], f32, tag="fp", name="fp")
                    for j in range(4):
                        nc.gpsimd.indirect_dma_start(
                            out=fp[:, j * DH:(j + 1) * DH], out_offset=None,
                            in_=ogq,
                            in_offset=IndirectOffsetOnAxis(
                                ap=fidxt[:, t * 4 + j:t * 4 + j + 1], axis=0))
                    nc.vector.tensor_tensor(out=fp[:], in0=fp[:], in1=bct[:],
                                            op=OP.add)
                    nc.scalar.activation(out=fp[:], in_=fp[:], func=AF.Relu)
                    sqf = fb.tile([P, DIN], f32, tag="sqf", name="sqf")
                    nc.scalar.activation(out=sqf[:], in_=fp[:], func=AF.Square,
                                         accum_out=n2f[:, t:t + 1])
                    fts.append(fp)

                nfF = fcolp.tile([P, NBLK], f32, tag="nfF", name="nfF")
                nc.scalar.activation(out=nfF[:], in_=n2f[:], func=AF.Sqrt)
                nc.vector.tensor_scalar_max(out=nfF[:], in0=nfF[:], scalar1=MIN_NORM)
                tfF = fcolp.tile([P, NBLK], f32, tag="tfF", name="tfF")
                nc.scalar.activation(out=tfF[:], in_=nfF[:], func=AF.Tanh)
                nc.vector.tensor_scalar_min(out=tfF[:], in0=tfF[:],
                                            scalar1=1 - PROJ_EPS)
                nc.vector.reciprocal(out=nfF[:], in_=nfF[:])
                sfF = fcolp.tile([P, NBLK], f32, tag="sfF", name="sfF")
                nc.vector.tensor_tensor(out=sfF[:], in0=tfF[:], in1=nfF[:], op=OP.mult)

                for t in range(NBLK):
                    yo = fb.tile([P, DIN], f16, tag="yo", name="yo")
                    nc.vector.tensor_scalar_mul(out=yo[:], in0=fts[t][:],
                                                scalar1=sfF[:, t:t + 1])
                    nc.sync.dma_start(out=y_out[t * P:(t + 1) * P, :], in_=yo[:])

    return nc


_CACHE = {}
_BIR_CACHE = os.path.expanduser("~/.cache/hgat_bir_v1.pkl")
_IN_NAMES = ("x_in", "w_in", "hb_in", "ai_in", "aj_in", "bc_in",
             "ed_in", "fidx_in")


class _FakeM:
    def __init__(self, arch):
        self.arch = arch


class _FakeNc:
    """Stand-in for a built Bass program: just enough surface for the
    bass_exec lowering (to_json_bytes / has_collectives / m.arch /
    target_bir_lowering)."""

    target_bir_lowering = False

    def __init__(self, bir, has_collectives, arch):
        self._bir = bir
        self.has_collectives = has_collectives
        self.m = _FakeM(arch)

    def to_json_bytes(self):
        return self._bir


def _bir_cache_load(key):
    import pickle
    try:
        with open(_BIR_CACHE, "rb") as f:
            blob = pickle.load(f)
        if blob.get("key") == key:
            return _FakeNc(blob["bir"], blob["has_collectives"], blob["arch"])
    except Exception:
        pass
    return None


def _bir_cache_save(key, nc):
    import pickle
    try:
        pn = nc.partition_id_tensor.name if nc.partition_id_tensor else None
        if pn != "partition_id":
            return
        blob = {"key": key, "bir": nc.to_json_bytes(),
                "has_collectives": nc.has_collectives, "arch": nc.m.arch}
        os.makedirs(os.path.dirname(_BIR_CACHE), exist_ok=True)
        tmp = _BIR_CACHE + ".tmp"
        with open(tmp, "wb") as f:
            pickle.dump(blob, f)
        os.replace(tmp, _BIR_CACHE)
    except Exception:
        pass


def _prepare_compiled(nc_like, n_groups):
    """Trace/lower/compile with abstract avals only (no input data needed),
    so it can run on a thread concurrently with host prep."""
    import jax
    import numpy as np_
    from jax.sharding import Mesh, PartitionSpec
    from jax.experimental.shard_map import shard_map
    from concourse import bass2jax

    try:
        jax.config.update("jax_compilation_cache_dir",
                          os.path.expanduser("~/.cache/jaxcache"))
    except Exception:
        pass
    bass2jax.install_neuronx_cc_hook()
    out_avals = (jax.core.ShapedArray((RPAD, DIN), np_.float16),)
    in_names = _IN_NAMES + ("partition_id",)
    n_params = len(_IN_NAMES)

    def _body(*args):
        operands = list(args)
        operands.append(bass2jax.partition_id_tensor())
        return tuple(bass2jax._bass_exec_p.bind(
            *operands, out_avals=out_avals, in_names=in_names,
            out_names=("y_out",), lowering_input_output_aliases=(),
            sim_require_finite=True, sim_require_nnan=True, nc=nc_like))

    devices = jax.devices()[:NC_]
    mesh = Mesh(np_.asarray(devices), ("core",))
    sharded = jax.jit(shard_map(
        _body, mesh=mesh, in_specs=(PartitionSpec("core"),) * n_params,
        out_specs=(PartitionSpec("core"),), check_rep=False),
        keep_unused=True)
    g = n_groups * 6 * IG
    specs = [
        jax.ShapeDtypeStruct((NC_ * RPAD, DIN), np_.float16),   # x_in
        jax.ShapeDtypeStruct((NC_ * DIN, DIN), np_.float32),    # w_in
        jax.ShapeDtypeStruct((NC_ * P, DIN), np_.float32),      # hb_in
        jax.ShapeDtypeStruct((NC_ * P, DIN), np_.float32),      # ai_in
        jax.ShapeDtypeStruct((NC_ * P, DIN), np_.float32),      # aj_in
        jax.ShapeDtypeStruct((NC_ * P, DIN), np_.float32),      # bc_in
        jax.ShapeDtypeStruct((NC_ * P, g), np_.int32),          # ed_in
        jax.ShapeDtypeStruct((NC_ * P, NBLK * 4), np_.int32),   # fidx_in
    ]
    t = time.time()
    lowered = sharded.lower(*specs)
    t = _tick("  lower", t)
    compiled = lowered.compile()
    _tick("  compile/load", t)
    return compiled


def _run_spmd(compiled, concat_in):
    import numpy as np_
    t = time.time()
    out = compiled(*concat_in)[0]
    out.block_until_ready()
    t = _tick("  execute", t)
    try:
        import concurrent.futures as cf
        res = np_.empty((NC_ * RPAD, DIN), np_.float16)

        def _pull(s):
            i0 = s.index[0].start or 0
            a = np_.asarray(s.data)
            res[i0:i0 + a.shape[0]] = a

        with cf.ThreadPoolExecutor(NC_) as ex:
            list(ex.map(_pull, out.addressable_shards))
    except Exception:
        res = np_.asarray(out)       # [NC_*RPAD, DIN] fp16
    _tick("  fetch", t)
    return res


def _device_kernel(x, W, b_lin, att, b_conv, ei):
    from concourse.bass_utils import run_bass_kernel_spmd

    t0 = time.time()
    x = np.ascontiguousarray(x, dtype=np.float32)
    W = np.asarray(W, dtype=np.float32)
    b_lin = np.asarray(b_lin, dtype=np.float32)
    att = np.asarray(att, dtype=np.float32)
    b_conv = np.asarray(b_conv, dtype=np.float32)

    hb = _proj(_expmap0(b_lin[None, :].astype(np.float64)))
    y2 = float((hb ** 2).sum())
    hb_b = np.tile(hb.astype(np.float32), (P, 1))
    wrhs = np.ascontiguousarray(W.T)
    bc_b = np.tile(b_conv[None, :], (P, 1)).astype(np.float32)

    # ---- edges: sort by (core, block), pad per block to x256 ----
    src = ei[0].astype(np.int32)
    dst = ei[1].astype(np.int32)
    loop = np.arange(N, dtype=np.int32)
    src = np.concatenate([src, loop])
    dst = np.concatenate([dst, loop])
    core = dst // RPC
    rem = dst - core * RPC
    bk = rem >> 7
    dl = rem - (bk << 7)
    key = core * NBLK + bk
    counts = np.bincount(key, minlength=NC_ * NBLK).reshape(NC_, NBLK)
    tpb2 = [max(1, int(np.ceil(counts[:, b].max() / 256.0))) for b in range(NBLK)]
    n_iters = int(sum(tpb2))
    epad = 256 * n_iters
    n_groups = (n_iters + IG - 1) // IG

    # program shape is now known: load/build the BIR and start the
    # executable load on a thread, overlapping the rest of host prep
    key_nc = (2, tuple(tpb2), epad, round(y2, 12))
    nc_like = _bir_cache_load(key_nc)
    if nc_like is None:
        if key_nc not in _CACHE:
            _CACHE.clear()
            _CACHE[key_nc] = _build_nc(tpb2, epad, y2)
        nc_like = _CACHE[key_nc]
        _bir_cache_save(key_nc, nc_like)
    import threading
    _holder = {}

    def _bg_compile():
        try:
            _holder["c"] = _prepare_compiled(nc_like, n_groups)
        except Exception as e:  # surfaced at join
            _holder["e"] = e

    _th = threading.Thread(target=_bg_compile)
    _th.start()

    order = np.argsort(key, kind="stable")
    starts = np.concatenate([[0], np.cumsum(counts.reshape(-1))]).astype(np.int64)
    blk_off = np.concatenate(
        [[0], np.cumsum(np.asarray(tpb2) * 256)]).astype(np.int64)

    src_s = src[order]
    dst_s = dst[order]
    dl_s = dl[order].astype(np.float32)

    # packed per-core edge stream: (src, dst_global, dstloc_f32bits)
    ed3 = np.empty((NC_, epad, 3), np.int32)
    ed3[:, :, 0] = PADROW
    ed3[:, :, 1] = 0
    ed3[:, :, 2] = np.float32(0.0).view(np.int32)
    for c in range(NC_):
        for b in range(NBLK):
            k = c * NBLK + b
            s0, s1 = starts[k], starts[k + 1]
            cnt = s1 - s0
            o0 = blk_off[b]
            ed3[c, o0:o0 + cnt, 0] = src_s[s0:s1]
            ed3[c, o0:o0 + cnt, 1] = dst_s[s0:s1]
            ed3[c, o0:o0 + cnt, 2] = dl_s[s0:s1].view(np.int32)
    # regroup into [n_groups, 128, 6*IG]; per-iter column layout
    # [src_lo, src_hi, dstg_lo, dstg_hi, dlf_lo, dlf_hi]
    ed4 = np.zeros((NC_, n_groups * IG, P, 6), np.int32)
    ed4[:, :, :, 0:2] = PADROW
    ed4[:, :, :, 4:6] = np.float32(0.0).view(np.int32)
    e5 = ed3.reshape(NC_, n_iters, 2, P, 3)
    ed4[:, :n_iters, :, 0] = e5[:, :, 0, :, 0]
    ed4[:, :n_iters, :, 1] = e5[:, :, 1, :, 0]
    ed4[:, :n_iters, :, 2] = e5[:, :, 0, :, 1]
    ed4[:, :n_iters, :, 3] = e5[:, :, 1, :, 1]
    ed4[:, :n_iters, :, 4] = e5[:, :, 0, :, 2]
    ed4[:, :n_iters, :, 5] = e5[:, :, 1, :, 2]
    ed4 = np.ascontiguousarray(
        ed4.reshape(NC_, n_groups * IG, P, 6).transpose(0, 2, 1, 3)
        .reshape(NC_, P, n_groups * IG * 6))

    # ---- per-core final-permutation quarter-row indices ----
    p_ar = np.arange(P, dtype=np.int64)
    b_ar = np.arange(NBLK, dtype=np.int64)
    fidx_all = []
    for c in range(NC_):
        r = c * RPC + b_ar[None, :, None] * P + p_ar[:, None, None]
        j = np.arange(4, dtype=np.int64)[None, None, :]
        f = 4 * r + j
        h = f // N
        n = f - h * N
        rp = (n // RPC) * RPAD + (n % RPC)
        q = rp * 4 + h
        q = np.where(r < N, q, 0)
        fidx_all.append(q.reshape(P, NBLK * 4).astype(np.int32))

    xpad = np.zeros((NPAD, DIN), np.float16)
    for c in range(NC_):
        xpad[c * RPAD:c * RPAD + RPC] = x[c * RPC:(c + 1) * RPC].astype(np.float16)
    t0 = _tick("host prep", t0)

    ai_g = np.empty((NC_ * P, DIN), np.float32)
    aj_g = np.empty((NC_ * P, DIN), np.float32)
    for c in range(NC_):
        hsel = c // 2
        ai_g[c * P:(c + 1) * P] = np.concatenate([att[hsel, :DH]] * 4)[None, :]
        aj_g[c * P:(c + 1) * P] = np.concatenate([att[hsel, DH:]] * 4)[None, :]
    globals_in = [
        xpad,
        np.tile(wrhs, (NC_, 1)),
        np.tile(hb_b, (NC_, 1)),
        ai_g,
        aj_g,
        np.tile(bc_b, (NC_, 1)),
        ed4.reshape(NC_ * P, -1),
        np.concatenate(fidx_all, axis=0),
    ]

    def _mk_in_maps():
        return [
            {n: g[c * (g.shape[0] // NC_):(c + 1) * (g.shape[0] // NC_)]
             for n, g in zip(_IN_NAMES, globals_in)}
            for c in range(NC_)
        ]

    try:
        _th.join()
        if "e" in _holder:
            raise _holder["e"]
        t0 = _tick("compile join", t0)
        flat = _run_spmd(_holder["c"], globals_in)
        t0 = _tick("spmd run", t0)
        out = np.empty((N, DIN), np.float32)
        for c in range(NC_):
            out[c * RPC:(c + 1) * RPC] = flat[c * RPAD:c * RPAD + RPC]
    except Exception:
        import traceback
        traceback.print_exc()
        if key_nc not in _CACHE:
            _CACHE.clear()
            _CACHE[key_nc] = _build_nc(tpb2, epad, y2)
        r = run_bass_kernel_spmd(_CACHE[key_nc], _mk_in_maps(),
                                 list(range(NC_)), trace=False)
        t0 = _tick("spmd run (fallback)", t0)
        out = np.concatenate(
            [r.results[c]["y_out"][:RPC] for c in range(NC_)], axis=0)
        out = out.astype(np.float32)
    _tick("gather out", t0)
    return out


# ---------------- host fallback (no scipy, slow but correct) --------------
def _host_kernel(x, W, b_lin, att, b_conv, ei):
    x = np.asarray(x, dtype=np.float32)
    xh = _proj(_expmap0(_logmap0(x) @ np.asarray(W, np.float32).T))
    hb = _proj(_expmap0(np.asarray(b_lin, np.float32)[None, :]))
    x2 = (xh * xh).sum(-1, keepdims=True)
    b2 = (hb * hb).sum(-1, keepdims=True)
    xy = (xh * hb).sum(-1, keepdims=True)
    numer = (1 + 2 * xy + b2) * xh + (1 - x2) * hb
    denom = np.clip(1 + 2 * xy + x2 * b2, MIN_NORM, None)
    xh = _proj(numer / denom)
    L = _logmap0(xh)
    Lf = L.reshape(-1)
    G = np.empty((N, H * DH), np.float32)
    for h in range(H):
        G[:, h * DH:(h + 1) * DH] = Lf[h * N * DH:(h + 1) * N * DH].reshape(N, DH)
    si = (G.reshape(N, H, DH) * att[None, :, :DH]).sum(-1).astype(np.float32)
    sj = (G.reshape(N, H, DH) * att[None, :, DH:]).sum(-1).astype(np.float32)
    loop = np.arange(N, dtype=np.int64)
    srcv = np.concatenate([ei[0], loop])
    dstv = np.concatenate([ei[1], loop])
    alpha = si[dstv] + sj[srcv]
    alpha = np.where(alpha > 0, alpha, np.float32(0.2) * alpha)
    w = np.exp(alpha)
    den = np.zeros((N, H), np.float32)
    for h in range(H):
        den[:, h] = np.bincount(dstv, weights=w[:, h], minlength=N)
    order = np.argsort(dstv, kind="stable")
    ds = dstv[order]
    seg = np.concatenate([[0], np.flatnonzero(np.diff(ds)) + 1])
    seg_ids = ds[seg]
    msg = G[srcv[order]].reshape(-1, H, DH) * w[order][:, :, None]
    sums = np.add.reduceat(msg.reshape(-1, H * DH), seg, axis=0)
    numg = np.zeros((N, H * DH), np.float32)
    numg[seg_ids] = sums
    outg = numg.reshape(N, H, DH) / den[:, :, None]
    final = outg.transpose(1, 0, 2).reshape(N, H * DH)
    final = final + np.asarray(b_conv, np.float32)
    final = np.maximum(final, 0.0)
    return _proj(_expmap0(final)).astype(np.float32)


def kernel(x, edge_index, W, b_lin, att, b_conv):
    ei = np.asarray(edge_index).astype(np.int64)
    try:
        return _device_kernel(x, W, b_lin, att, b_conv, ei)
    except Exception:
        import traceback
        traceback.print_exc()
        return _host_kernel(np.asarray(x), np.asarray(W), np.asarray(b_lin),
                            np.asarray(att), np.asarray(b_conv), ei)


# revision 30
# speedup vs baseline: 1.2001x; 1.2001x over previous
"""HGAT layer (hyperbolic GAT) kernel for trn2, 8 NeuronCores, full-device.

Single SPMD bass/Tile program on 8 cores:
  phase 1  per core: its 6250 rows of x through HypLinear
           (logmap0 -> @W.T -> expmap0 -> proj -> mobius+hb -> proj ->
           logmap0) -> L rows + per-64-chunk dots with att (vLi/vLj).
           Column math ([128,1] scalars) is batched across the 49 row
           tiles into [128,49] pipelines.
  AllGather L|vLi|vLj.
  phase 2  strided-DMA permute into Gmat row order (the reference's
           view(heads,-1,DH) quirk): gather table T=[sj|Gmat] and
           si_full.
  phase 3  edge-parallel segment softmax. Edges sorted by dst and
           grouped into 128-slot dst blocks (padded per block). Per 256
           edges: indirect-gather T[src] rows and si_full[dst] rows,
           build one-hot(dst_local) vs an iota row, w =
           exp(leaky_relu(si+sj, 0.2)), PSUM-accumulate one-hot.T @
           [w | w*msg] -> per-node denominator and weighted sum.
  AllGather aggregates.
  phase 4  inverse permutation via indexed quarter-row gathers, +b_conv,
           relu, expmap0, proj -> output (fp16 download).

Host only sorts/pads the edge list and packs small constants; all
core-dependent addressing is index DATA uploaded per core, so one
program serves all cores.
"""
import os
import time
import numpy as np

N, E, DIN, H, DH = 50000, 800000, 256, 4, 64
MIN_NORM = 1e-15
PROJ_EPS = 4e-3
P = 128
RPC = 6250            # real rows per core
RPAD = 6272           # padded rows per core (49 tiles)
NBLK = RPAD // P      # 49 dst blocks per core
NC_ = 8
NPAD = NC_ * RPAD     # 50176
PADROW = N            # gather-table pad row index
TW = 260              # table row: [sj(4) | msg(256)]
IG = 4                # edge iters per index-DMA group

_TIME = os.environ.get("HGAT_TIME") == "1"


def _tick(tag, t0):
    if _TIME:
        print(f"  [hgat] {tag}: {time.time()-t0:.3f}s", flush=True)
    return time.time()


# ---------------- host-side hyperbolic helpers (small tensors only) -------
def _norm(a):
    return np.clip(np.linalg.norm(a, axis=-1, keepdims=True), MIN_NORM, None)


def _logmap0(a):
    n = _norm(a)
    return np.arctanh(np.minimum(n, 1 - 1e-7)) * a / n


def _expmap0(u):
    n = _norm(u)
    return np.tanh(n) * u / n


def _proj(a):
    n = _norm(a)
    mx = 1.0 - PROJ_EPS
    return np.where(n > mx, a / n * mx, a)


# ---------------- tile framework patches (walrus: 1 sem wait / inst) ------
def _install_tile_patches():
    import concourse.tile as tile
    from concourse import mybir
    from concourse.vector_clock import ScopedClock, VectorClock

    if getattr(tile.TileContext, "_hgat_patched", False):
        return

    def _drain_and_barrier_split(self, tick_clock, wait_clock):
        gc = tick_clock.global_clock
        n = len(gc)
        for i in range(n):
            t = gc[i]
            if t <= 0:
                continue
            vc = VectorClock([0] * n)
            vc.require_at_least(i, t)
            d = self.nc.sync.drain()
            wait_clock.add_sem_waits(d.ins, ScopedClock({None: vc}))
        self.nc.all_engine_barrier()
        assert self.sems is not None
        popped = self.nc._tile_sem_poison_stack.pop()
        assert popped is self._sem_poison
        self.nc.clear_and_free_semaphores(list(self.sems.allocated().values()))
        self.nc.all_engine_barrier()

    _orig_add = tile.TileContext._add_instruction

    def _add_instruction_split(self, inst):
        si = inst.sync_info
        if si is not None and len(si.on_wait) > 1:
            waits = list(si.on_wait)
            for k, w in enumerate(waits[:-1]):
                carrier = mybir.InstEventSemaphore(
                    name=f"{inst.name}-sw{k}",
                    engine=inst.engine,
                    ins=[],
                    outs=[],
                    sync_info=mybir.SyncInfo(on_wait=[w], on_update=[]),
                )
                _orig_add(self, carrier)
            inst.sync_info = mybir.SyncInfo(
                on_wait=[waits[-1]], on_update=list(si.on_update)
            )
        return _orig_add(self, inst)

    tile.TileContext._drain_and_barrier = _drain_and_barrier_split
    tile.TileContext._add_instruction = _add_instruction_split
    tile.TileContext._hgat_patched = True


# ---------------- bass program ------------------------------------------
def _build_nc(tpb2, epad, y2):
    """tpb2: per dst block, number of 256-edge iterations (same all cores)."""
    from concourse import bass, mybir
    from concourse.bass import IndirectOffsetOnAxis
    from concourse.masks import make_identity
    import concourse.tile as tile

    _install_tile_patches()

    f32 = mybir.dt.float32
    f16 = mybir.dt.float16
    i32 = mybir.dt.int32
    AF = mybir.ActivationFunctionType
    AX = mybir.AxisListType
    OP = mybir.AluOpType

    n_iters = int(sum(tpb2))
    n_groups = (n_iters + IG - 1) // IG

    nc = bass.Bass("TRN2", target_bir_lowering=False, debug=False, num_devices=NC_)
    x_in = nc.dram_tensor("x_in", [RPAD, DIN], f16, kind="ExternalInput")
    w_in = nc.dram_tensor("w_in", [DIN, DIN], f32, kind="ExternalInput")
    hb_in = nc.dram_tensor("hb_in", [P, DIN], f32, kind="ExternalInput")
    ai_in = nc.dram_tensor("ai_in", [P, DIN], f32, kind="ExternalInput")
    aj_in = nc.dram_tensor("aj_in", [P, DIN], f32, kind="ExternalInput")
    bc_in = nc.dram_tensor("bc_in", [P, DIN], f32, kind="ExternalInput")
    ed_in = nc.dram_tensor("ed_in", [P, n_groups * 6 * IG], i32, kind="ExternalInput")
    fidx_in = nc.dram_tensor("fidx_in", [P, NBLK * 4], i32, kind="ExternalInput")
    y_out = nc.dram_tensor("y_out", [RPAD, DIN], f16, kind="ExternalOutput")

    with tile.TileContext(nc) as tc:
        with (
            tc.tile_pool(name="dram", bufs=1, space="DRAM") as dram,
            tc.tile_pool(name="const", bufs=1) as cp,
        ):
            lv_loc = dram.tile([RPAD, DIN + 8], f32, tag="lv_loc", name="lv_loc")
            lv_full = dram.tile([NPAD, DIN + 8], f32, tag="lv_full", name="lv_full")
            t_tab = dram.tile([N + 1, TW], f32, tag="t_tab", name="t_tab")
            si_full = dram.tile([NPAD, 4], f32, tag="si_full", name="si_full")
            og_loc = dram.tile([RPAD, DIN], f32, tag="og_loc", name="og_loc")
            og_full = dram.tile([NPAD, DIN], f32, tag="og_full", name="og_full")

            ident = cp.tile([P, P], f32, tag="id", name="ident")
            make_identity(nc, ident[:])
            iota_i = cp.tile([P, P], i32, tag="ioi", name="iota_i")
            nc.gpsimd.iota(iota_i[:], pattern=[[1, P]], base=0, channel_multiplier=0)
            iota_row = cp.tile([P, P], f32, tag="ior", name="iota_row")
            nc.vector.tensor_copy(out=iota_row[:], in_=iota_i[:])
            w0 = cp.tile([P, DIN], f32, tag="w0", name="w0")
            w1 = cp.tile([P, DIN], f32, tag="w1", name="w1")
            nc.sync.dma_start(out=w0[:], in_=w_in[0:P, :])
            nc.sync.dma_start(out=w1[:], in_=w_in[P:DIN, :])
            hbt = cp.tile([P, DIN], f32, tag="hb", name="hbt")
            nc.sync.dma_start(out=hbt[:], in_=hb_in[:])
            ait = cp.tile([P, DIN], f32, tag="ai", name="ait")
            nc.sync.dma_start(out=ait[:], in_=ai_in[:])
            ajt = cp.tile([P, DIN], f32, tag="aj", name="ajt")
            nc.sync.dma_start(out=ajt[:], in_=aj_in[:])
            bct = cp.tile([P, DIN], f32, tag="bc", name="bct")
            nc.sync.dma_start(out=bct[:], in_=bc_in[:])
            fidxt = cp.tile([P, NBLK * 4], i32, tag="fidx", name="fidxt")
            nc.sync.dma_start(out=fidxt[:], in_=fidx_in[:])

            # ======== phase 1: dense (column math batched over tiles) =====
            with (
                tc.tile_pool(name="p1y", bufs=NBLK) as ypool,
                tc.tile_pool(name="p1h", bufs=NBLK) as hpool,
                tc.tile_pool(name="p1sb", bufs=3) as sb,
                tc.tile_pool(name="p1c", bufs=2) as colp,
                tc.tile_pool(name="p1ps", bufs=2, space="PSUM") as ps,
                tc.tile_pool(name="p1py", bufs=2, space="PSUM") as py,
            ):
                def cols(tag):
                    return colp.tile([P, NBLK], f32, tag=tag, name=tag)

                n2c = cols("n2c")
                for t in range(NBLK):
                    xt = sb.tile([P, DIN], f16, tag="x", name="xt")
                    nc.sync.dma_start(out=xt[:], in_=x_in[t * P:(t + 1) * P, :])
                    sq = sb.tile([P, DIN], f32, tag="sq", name="sq")
                    nc.scalar.activation(out=sq[:], in_=xt[:], func=AF.Square,
                                         accum_out=n2c[:, t:t + 1])

                # pipeline A: s = atanh(min(max(sqrt(n2),MN),1-eps)) / n
                nA = cols("nA")
                nc.scalar.activation(out=nA[:], in_=n2c[:], func=AF.Sqrt)
                nc.vector.tensor_scalar_max(out=nA[:], in0=nA[:], scalar1=MIN_NORM)
                nmA = cols("nmA")
                nc.vector.tensor_scalar_min(out=nmA[:], in0=nA[:], scalar1=1 - 1e-7)
                a1A = cols("a1A")
                nc.vector.tensor_scalar_add(out=a1A[:], in0=nmA[:], scalar1=1.0)
                a2A = cols("a2A")
                nc.vector.tensor_scalar(out=a2A[:], in0=nmA[:], scalar1=-1.0,
                                        scalar2=1.0, op0=OP.mult, op1=OP.add)
                nc.vector.reciprocal(out=a2A[:], in_=a2A[:])
                nc.vector.tensor_tensor(out=a1A[:], in0=a1A[:], in1=a2A[:], op=OP.mult)
                nc.scalar.activation(out=a1A[:], in_=a1A[:], func=AF.Ln)
                nc.vector.reciprocal(out=nA[:], in_=nA[:])
                sA = cols("sA")
                nc.vector.tensor_tensor(out=sA[:], in0=a1A[:], in1=nA[:], op=OP.mult)
                nc.vector.tensor_scalar_mul(out=sA[:], in0=sA[:], scalar1=0.5)

                # pass 2: matmul + expmap0 norm (x re-streamed from DRAM)
                n2y = cols("n2y")
                yts = []
                for t in range(NBLK):
                    xt = sb.tile([P, DIN], f16, tag="x", name="xt")
                    nc.sync.dma_start(out=xt[:], in_=x_in[t * P:(t + 1) * P, :])
                    lx = sb.tile([P, DIN], f32, tag="lx", name="lx")
                    nc.vector.tensor_scalar_mul(out=lx[:], in0=xt[:],
                                                scalar1=sA[:, t:t + 1])
                    yp = py.tile([P, DIN], f32, tag="yp", name="yp")
                    for c in range(2):
                        tp = ps.tile([P, P], f32, tag="tp", name="tp")
                        nc.tensor.transpose(out=tp[:], in_=lx[:, c * P:(c + 1) * P],
                                            identity=ident[:])
                        lxT = sb.tile([P, P], f32, tag="lxT", name="lxT")
                        nc.scalar.copy(out=lxT[:], in_=tp[:])
                        nc.tensor.matmul(out=yp[:], lhsT=lxT[:],
                                         rhs=(w0 if c == 0 else w1)[:],
                                         start=(c == 0), stop=(c == 1))
                    sq2 = sb.tile([P, DIN], f32, tag="sq", name="sq2")
                    nc.scalar.activation(out=sq2[:], in_=yp[:], func=AF.Square,
                                         accum_out=n2y[:, t:t + 1])
                    yt = ypool.tile([P, DIN], f32, tag="y", name="yt")
                    nc.vector.tensor_copy(out=yt[:], in_=yp[:])
                    yts.append(yt)

                # pipeline B: s2 = min(tanh(ny),1-eps)/ny ; x2 = min(..)^2
                nyB = cols("nyB")
                nc.scalar.activation(out=nyB[:], in_=n2y[:], func=AF.Sqrt)
                nc.vector.tensor_scalar_max(out=nyB[:], in0=nyB[:], scalar1=MIN_NORM)
                thB = cols("thB")
                nc.scalar.activation(out=thB[:], in_=nyB[:], func=AF.Tanh)
                nc.vector.tensor_scalar_min(out=thB[:], in0=thB[:],
                                            scalar1=1 - PROJ_EPS)
                nc.vector.reciprocal(out=nyB[:], in_=nyB[:])
                s2B = cols("s2B")
                nc.vector.tensor_tensor(out=s2B[:], in0=thB[:], in1=nyB[:], op=OP.mult)
                x2B = cols("x2B")
                nc.vector.tensor_tensor(out=x2B[:], in0=thB[:], in1=thB[:], op=OP.mult)

                # pass 3: mobius dot products
                xyC = cols("xyC")
                hts = []
                for t in range(NBLK):
                    xh1 = hpool.tile([P, DIN], f32, tag="xh1", name="xh1")
                    nc.vector.tensor_scalar_mul(out=xh1[:], in0=yts[t][:],
                                                scalar1=s2B[:, t:t + 1])
                    mm = sb.tile([P, DIN], f32, tag="mm", name="mm")
                    nc.vector.tensor_tensor(out=mm[:], in0=xh1[:], in1=hbt[:],
                                            op=OP.mult)
                    nc.vector.reduce_sum(out=xyC[:, t:t + 1], in_=mm[:], axis=AX.X)
                    hts.append(xh1)

                # pipeline C: mobius scalar chain
                a1m = cols("a1m")
                nc.vector.tensor_scalar(out=a1m[:], in0=xyC[:], scalar1=2.0,
                                        scalar2=1.0 + y2, op0=OP.mult, op1=OP.add)
                a2m = cols("a2m")
                nc.vector.tensor_scalar(out=a2m[:], in0=x2B[:], scalar1=-1.0,
                                        scalar2=1.0, op0=OP.mult, op1=OP.add)
                d1m = cols("d1m")
                nc.vector.tensor_scalar(out=d1m[:], in0=x2B[:], scalar1=y2,
                                        scalar2=1.0, op0=OP.mult, op1=OP.add)
                denm = cols("denm")
                nc.vector.tensor_scalar(out=denm[:], in0=xyC[:], scalar1=2.0,
                                        scalar2=0.0, op0=OP.mult, op1=OP.add)
                nc.vector.tensor_tensor(out=denm[:], in0=denm[:], in1=d1m[:], op=OP.add)
                nc.vector.reciprocal(out=denm[:], in_=denm[:])

                # pass 4: xh2 = (a1m*xh1 + a2m*hb) * rden ; its norm
                n2x = cols("n2x")
                for t in range(NBLK):
                    xh1 = hts[t]
                    t1 = sb.tile([P, DIN], f32, tag="t1", name="t1")
                    nc.vector.tensor_scalar_mul(out=t1[:], in0=xh1[:],
                                                scalar1=a1m[:, t:t + 1])
                    t2 = sb.tile([P, DIN], f32, tag="t2", name="t2")
                    nc.vector.tensor_scalar_mul(out=t2[:], in0=hbt[:],
                                                scalar1=a2m[:, t:t + 1])
                    nc.vector.tensor_tensor(out=t1[:], in0=t1[:], in1=t2[:], op=OP.add)
                    # reuse xh1 slot for xh2
                    nc.vector.tensor_scalar_mul(out=xh1[:], in0=t1[:],
                                                scalar1=denm[:, t:t + 1])
                    sq3 = sb.tile([P, DIN], f32, tag="sq3", name="sq3")
                    nc.scalar.activation(out=sq3[:], in_=xh1[:], func=AF.Square,
                                         accum_out=n2x[:, t:t + 1])

                # pipeline D: s3 = atanh(min(max(sqrt,MN),1-PROJ_EPS)) / n
                nxD = cols("nxD")
                nc.scalar.activation(out=nxD[:], in_=n2x[:], func=AF.Sqrt)
                nc.vector.tensor_scalar_max(out=nxD[:], in0=nxD[:], scalar1=MIN_NORM)
                ncD = cols("ncD")
                nc.vector.tensor_scalar_min(out=ncD[:], in0=nxD[:],
                                            scalar1=1 - PROJ_EPS)
                b1D = cols("b1D")
                nc.vector.tensor_scalar_add(out=b1D[:], in0=ncD[:], scalar1=1.0)
                b2D = cols("b2D")
                nc.vector.tensor_scalar(out=b2D[:], in0=ncD[:], scalar1=-1.0,
                                        scalar2=1.0, op0=OP.mult, op1=OP.add)
                nc.vector.reciprocal(out=b2D[:], in_=b2D[:])
                nc.vector.tensor_tensor(out=b1D[:], in0=b1D[:], in1=b2D[:], op=OP.mult)
                nc.scalar.activation(out=b1D[:], in_=b1D[:], func=AF.Ln)
                nc.vector.reciprocal(out=nxD[:], in_=nxD[:])
                s3D = cols("s3D")
                nc.vector.tensor_tensor(out=s3D[:], in0=b1D[:], in1=nxD[:], op=OP.mult)
                nc.vector.tensor_scalar_mul(out=s3D[:], in0=s3D[:], scalar1=0.5)

                # pass 5: L, vLi, vLj, store
                for t in range(NBLK):
                    lv = sb.tile([P, DIN + 8], f32, tag="lv", name="lv")
                    nc.vector.tensor_scalar_mul(out=lv[:, 0:DIN], in0=hts[t][:],
                                                scalar1=s3D[:, t:t + 1])
                    mi = sb.tile([P, DIN], f32, tag="mi", name="mi")
                    nc.vector.tensor_tensor(out=mi[:], in0=lv[:, 0:DIN], in1=ait[:],
                                            op=OP.mult)
                    mj = sb.tile([P, DIN], f32, tag="mj", name="mj")
                    nc.vector.tensor_tensor(out=mj[:], in0=lv[:, 0:DIN], in1=ajt[:],
                                            op=OP.mult)
                    nc.vector.reduce_sum(
                        out=lv[:, DIN:DIN + 4],
                        in_=mi[:].rearrange("p (h d) -> p h d", h=4), axis=AX.X)
                    nc.vector.reduce_sum(
                        out=lv[:, DIN + 4:DIN + 8],
                        in_=mj[:].rearrange("p (h d) -> p h d", h=4), axis=AX.X)
                    nc.sync.dma_start(out=lv_loc[t * P:(t + 1) * P, :], in_=lv[:])

            # ======== AllGather L|vLi|vLj =================================
            nc.gpsimd.collective_compute(
                "AllGather", mybir.AluOpType.bypass,
                replica_groups=[list(range(NC_))],
                ins=[lv_loc.opt()], outs=[lv_full.opt()])

            # ======== phase 2: permutations ===============================
            # T[i, 4 + h*64 + d] = L[node h*12500 + i//4, (i%4)*64 + d]
            for h in range(H):
                for q in range(2):
                    r0 = (2 * h + q) * RPAD
                    src = lv_full[r0:r0 + RPC, 0:DIN].rearrange(
                        "a (b d) -> a b d", b=4)
                    dst = t_tab[q * 25000:(q + 1) * 25000,
                                4 + h * DH:4 + (h + 1) * DH
                                ].rearrange("(a b) d -> a b d", b=4)
                    nc.sync.dma_start(out=dst, in_=src)
                    srcj = lv_full[r0:r0 + RPC, DIN + 4:DIN + 8]
                    dstj = t_tab[q * 25000:(q + 1) * 25000, h:h + 1
                                 ].rearrange("(a b) one -> a (b one)", b=4)
                    nc.sync.dma_start(out=dstj, in_=srcj)
                    srci = lv_full[r0:r0 + RPC, DIN:DIN + 4]
                    dsti = si_full[q * 25000:(q + 1) * 25000, h:h + 1
                                   ].rearrange("(a b) one -> a (b one)", b=4)
                    nc.sync.dma_start(out=dsti, in_=srci)
            padt = cp.tile([P, TW], f32, tag="padt", name="padt")
            nc.vector.memset(padt[:], 0.0)
            nc.vector.memset(padt[0:1, 0:4], -1000.0)
            nc.sync.dma_start(out=t_tab[N:N + 1, :], in_=padt[0:1, :])

            # ======== phase 3: edge loop ==================================
            with (
                tc.tile_pool(name="esb", bufs=4) as eb,
                tc.tile_pool(name="eid", bufs=1) as eip,
                tc.tile_pool(name="eac", bufs=2, space="PSUM") as eac,
            ):
                idxg = eip.tile([P, n_groups * 6 * IG], i32, tag="idxg",
                                name="idxg")
                nc.sync.dma_start(out=idxg[:], in_=ed_in[:])
                it = 0
                for b0 in range(NBLK):
                    niter = tpb2[b0]
                    acc = eac.tile([P, TW], f32, tag="acc", name="acc")
                    for k in range(niter):
                        # per-iter cols: [src_lo, src_hi, dstg_lo,
                        #                 dstg_hi, dlf_lo, dlf_hi]
                        c0 = it * 6
                        g2 = eb.tile([P, 2 * TW], f32, tag="g2", name="g2")
                        sie = eb.tile([P, 8], f32, tag="sie", name="sie")
                        for half in range(2):
                            nc.gpsimd.indirect_dma_start(
                                out=g2[:, half * TW:(half + 1) * TW],
                                out_offset=None, in_=t_tab[:],
                                in_offset=IndirectOffsetOnAxis(
                                    ap=idxg[:, c0 + half:c0 + half + 1],
                                    axis=0))
                            nc.gpsimd.indirect_dma_start(
                                out=sie[:, half * 4:(half + 1) * 4],
                                out_offset=None, in_=si_full[:],
                                in_offset=IndirectOffsetOnAxis(
                                    ap=idxg[:, c0 + 2 + half:c0 + 3 + half],
                                    axis=0))
                        dstf = idxg[:, c0 + 4:c0 + 6].bitcast(f32)
                        oh = eb.tile([P, 2 * P], f32, tag="oh", name="oh")
                        nc.vector.tensor_tensor(
                            out=oh[:].rearrange("p (t q) -> p t q", t=2),
                            in0=dstf[:, :, None].to_broadcast([P, 2, P]),
                            in1=iota_row[:, None, :].to_broadcast([P, 2, P]),
                            op=OP.is_equal)
                        nc.vector.tensor_tensor(
                            out=sie[:].rearrange("p (t c) -> p t c", t=2),
                            in0=sie[:].rearrange("p (t c) -> p t c", t=2),
                            in1=g2[:].rearrange("p (t c) -> p t c", t=2)[:, :, 0:4],
                            op=OP.add)
                        nc.scalar.activation(out=sie[:], in_=sie[:],
                                             func=AF.Prelu, alpha=0.2)
                        rhs = eb.tile([P, 2 * TW], f32, tag="rhs", name="rhs")
                        nc.scalar.activation(
                            out=rhs[:].rearrange("p (t c) -> p t c", t=2)[:, :, 0:4],
                            in_=sie[:].rearrange("p (t c) -> p t c", t=2),
                            func=AF.Exp)
                        nc.vector.tensor_tensor(
                            out=rhs[:].rearrange("p (t c) -> p t c", t=2)[:, :, 4:TW
                                ].rearrange("p t (h d) -> p t h d", h=4),
                            in0=g2[:].rearrange("p (t c) -> p t c", t=2)[:, :, 4:TW
                                ].rearrange("p t (h d) -> p t h d", h=4),
                            in1=rhs[:].rearrange("p (t c) -> p t c", t=2)[:, :, 0:4
                                ][:, :, :, None].to_broadcast([P, 2, 4, DH]),
                            op=OP.mult)
                        for half in range(2):
                            nc.tensor.matmul(
                                out=acc[:],
                                lhsT=oh[:, half * P:(half + 1) * P],
                                rhs=rhs[:, half * TW:(half + 1) * TW],
                                start=(k == 0 and half == 0),
                                stop=(k == niter - 1 and half == 1))
                        it += 1
                    rden4 = eb.tile([P, 4], f32, tag="rden4", name="rden4")
                    nc.vector.reciprocal(out=rden4[:], in_=acc[:, 0:4])
                    og = eb.tile([P, DIN], f32, tag="og", name="og")
                    nc.vector.tensor_tensor(
                        out=og[:].rearrange("p (h d) -> p h d", h=4),
                        in0=acc[:, 4:TW].rearrange("p (h d) -> p h d", h=4),
                        in1=rden4[:, :, None].to_broadcast([P, 4, DH]), op=OP.mult)
                    nc.sync.dma_start(out=og_loc[b0 * P:(b0 + 1) * P, :], in_=og[:])

            # ======== AllGather aggregates ================================
            nc.gpsimd.collective_compute(
                "AllGather", mybir.AluOpType.bypass,
                replica_groups=[list(range(NC_))],
                ins=[og_loc.opt()], outs=[og_full.opt()])

            # ======== phase 4: inverse permutation + final ================
            with (
                tc.tile_pool(name="fsb", bufs=3) as fb,
                tc.tile_pool(name="ffp", bufs=NBLK) as fpp,
                tc.tile_pool(name="fc", bufs=2) as fcolp,
            ):
                ogq = og_full[:].rearrange("n (q d) -> (n q) d", q=4)

                n2f = fcolp.tile([P, NBLK], f32, tag="n2f", name="n2f")
                fts = []
                for t in range(NBLK):
                    fp = fpp.tile([P, DIN], f32, tag="fp", name="fp")
                    for j in range(4):
                        nc.gpsimd.indirect_dma_start(
                            out=fp[:, j * DH:(j + 1) * DH], out_offset=None,
                            in_=ogq,
                            in_offset=IndirectOffsetOnAxis(
                                ap=fidxt[:, t * 4 + j:t * 4 + j + 1], axis=0))
                    nc.vector.tensor_tensor(out=fp[:], in0=fp[:], in1=bct[:],
                                            op=OP.add)
                    nc.scalar.activation(out=fp[:], in_=fp[:], func=AF.Relu)
                    sqf = fb.tile([P, DIN], f32, tag="sqf", name="sqf")
                    nc.scalar.activation(out=sqf[:], in_=fp[:], func=AF.Square,
                                         accum_out=n2f[:, t:t + 1])
                    fts.append(fp)

                nfF = fcolp.tile([P, NBLK], f32, tag="nfF", name="nfF")
                nc.scalar.activation(out=nfF[:], in_=n2f[:], func=AF.Sqrt)
                nc.vector.tensor_scalar_max(out=nfF[:], in0=nfF[:], scalar1=MIN_NORM)
                tfF = fcolp.tile([P, NBLK], f32, tag="tfF", name="tfF")
                nc.scalar.activation(out=tfF[:], in_=nfF[:], func=AF.Tanh)
                nc.vector.tensor_scalar_min(out=tfF[:], in0=tfF[:],
                                            scalar1=1 - PROJ_EPS)
                nc.vector.reciprocal(out=nfF[:], in_=nfF[:])
                sfF = fcolp.tile([P, NBLK], f32, tag="sfF", name="sfF")
                nc.vector.tensor_tensor(out=sfF[:], in0=tfF[:], in1=nfF[:], op=OP.mult)

                for t in range(NBLK):
                    yo = fb.tile([P, DIN], f16, tag="yo", name="yo")
                    nc.vector.tensor_scalar_mul(out=yo[:], in0=fts[t][:],
                                                scalar1=sfF[:, t:t + 1])
                    nc.sync.dma_start(out=y_out[t * P:(t + 1) * P, :], in_=yo[:])

    return nc


_CACHE = {}
_BIR_CACHE = os.path.expanduser("~/.cache/hgat_bir_v1.pkl")
_IN_NAMES = ("x_in", "w_in", "hb_in", "ai_in", "aj_in", "bc_in",
             "ed_in", "fidx_in")


class _FakeM:
    def __init__(self, arch):
        self.arch = arch


class _FakeNc:
    """Stand-in for a built Bass program: just enough surface for the
    bass_exec lowering (to_json_bytes / has_collectives / m.arch /
    target_bir_lowering)."""

    target_bir_lowering = False

    def __init__(self, bir, has_collectives, arch):
        self._bir = bir
        self.has_collectives = has_collectives
        self.m = _FakeM(arch)

    def to_json_bytes(self):
        return self._bir


def _bir_cache_load(key):
    import pickle
    try:
        with open(_BIR_CACHE, "rb") as f:
            blob = pickle.load(f)
        if blob.get("key") == key:
            return _FakeNc(blob["bir"], blob["has_collectives"], blob["arch"])
    except Exception:
        pass
    return None


def _bir_cache_save(key, nc):
    import pickle
    try:
        pn = nc.partition_id_tensor.name if nc.partition_id_tensor else None
        if pn != "partition_id":
            return
        blob = {"key": key, "bir": nc.to_json_bytes(),
                "has_collectives": nc.has_collectives, "arch": nc.m.arch}
        os.makedirs(os.path.dirname(_BIR_CACHE), exist_ok=True)
        tmp = _BIR_CACHE + ".tmp"
        with open(tmp, "wb") as f:
            pickle.dump(blob, f)
        os.replace(tmp, _BIR_CACHE)
    except Exception:
        pass


def _prepare_compiled(nc_like, n_groups):
    """Trace/lower/compile with abstract avals only (no input data needed),
    so it can run on a thread concurrently with host prep."""
    import jax
    import numpy as np_
    from jax.sharding import Mesh, PartitionSpec
    from jax.experimental.shard_map import shard_map
    from concourse import bass2jax

    try:
        jax.config.update("jax_compilation_cache_dir",
                          os.path.expanduser("~/.cache/jaxcache"))
    except Exception:
        pass
    bass2jax.install_neuronx_cc_hook()
    out_avals = (jax.core.ShapedArray((RPAD, DIN), np_.float16),)
    in_names = _IN_NAMES + ("partition_id",)
    n_params = len(_IN_NAMES)

    def _body(*args):
        operands = list(args)
        operands.append(bass2jax.partition_id_tensor())
        return tuple(bass2jax._bass_exec_p.bind(
            *operands, out_avals=out_avals, in_names=in_names,
            out_names=("y_out",), lowering_input_output_aliases=(),
            sim_require_finite=True, sim_require_nnan=True, nc=nc_like))

    devices = jax.devices()[:NC_]
    mesh = Mesh(np_.asarray(devices), ("core",))
    sharded = jax.jit(shard_map(
        _body, mesh=mesh, in_specs=(PartitionSpec("core"),) * n_params,
        out_specs=(PartitionSpec("core"),), check_rep=False),
        keep_unused=True)
    g = n_groups * 6 * IG
    specs = [
        jax.ShapeDtypeStruct((NC_ * RPAD, DIN), np_.float16),   # x_in
        jax.ShapeDtypeStruct((NC_ * DIN, DIN), np_.float32),    # w_in
        jax.ShapeDtypeStruct((NC_ * P, DIN), np_.float32),      # hb_in
        jax.ShapeDtypeStruct((NC_ * P, DIN), np_.float32),      # ai_in
        jax.ShapeDtypeStruct((NC_ * P, DIN), np_.float32),      # aj_in
        jax.ShapeDtypeStruct((NC_ * P, DIN), np_.float32),      # bc_in
        jax.ShapeDtypeStruct((NC_ * P, g), np_.int32),          # ed_in
        jax.ShapeDtypeStruct((NC_ * P, NBLK * 4), np_.int32),   # fidx_in
    ]
    t = time.time()
    lowered = sharded.lower(*specs)
    t = _tick("  lower", t)
    compiled = lowered.compile()
    _tick("  compile/load", t)
    return compiled


def _run_spmd(compiled, concat_in):
    import numpy as np_
    t = time.time()
    out = compiled(*concat_in)[0]
    out.block_until_ready()
    t = _tick("  execute", t)
    try:
        import concurrent.futures as cf
        res = np_.empty((NC_ * RPAD, DIN), np_.float16)

        def _pull(s):
            i0 = s.index[0].start or 0
            a = np_.asarray(s.data)
            res[i0:i0 + a.shape[0]] = a

        with cf.ThreadPoolExecutor(NC_) as ex:
            list(ex.map(_pull, out.addressable_shards))
    except Exception:
        res = np_.asarray(out)       # [NC_*RPAD, DIN] fp16
    _tick("  fetch", t)
    return res


def _device_kernel(x, W, b_lin, att, b_conv, ei):
    from concourse.bass_utils import run_bass_kernel_spmd
    import threading

    def _warm_backend():
        try:
            import jax
            jax.devices()
        except Exception:
            pass

    _wt = threading.Thread(target=_warm_backend, daemon=True)
    _wt.start()

    t0 = time.time()
    x = np.ascontiguousarray(x, dtype=np.float32)
    W = np.asarray(W, dtype=np.float32)
    b_lin = np.asarray(b_lin, dtype=np.float32)
    att = np.asarray(att, dtype=np.float32)
    b_conv = np.asarray(b_conv, dtype=np.float32)

    hb = _proj(_expmap0(b_lin[None, :].astype(np.float64)))
    y2 = float((hb ** 2).sum())
    hb_b = np.tile(hb.astype(np.float32), (P, 1))
    wrhs = np.ascontiguousarray(W.T)
    bc_b = np.tile(b_conv[None, :], (P, 1)).astype(np.float32)

    # ---- edges: sort by (core, block), pad per block to x256 ----
    src = ei[0].astype(np.int32)
    dst = ei[1].astype(np.int32)
    loop = np.arange(N, dtype=np.int32)
    src = np.concatenate([src, loop])
    dst = np.concatenate([dst, loop])
    core = dst // RPC
    rem = dst - core * RPC
    bk = rem >> 7
    dl = rem - (bk << 7)
    key = core * NBLK + bk
    counts = np.bincount(key, minlength=NC_ * NBLK).reshape(NC_, NBLK)
    tpb2 = [max(1, int(np.ceil(counts[:, b].max() / 256.0))) for b in range(NBLK)]
    n_iters = int(sum(tpb2))
    epad = 256 * n_iters
    n_groups = (n_iters + IG - 1) // IG

    # program shape is now known: load/build the BIR and start the
    # executable load on a thread, overlapping the rest of host prep
    key_nc = (2, tuple(tpb2), epad, round(y2, 12))
    nc_like = _bir_cache_load(key_nc)
    if nc_like is None:
        if key_nc not in _CACHE:
            _CACHE.clear()
            _CACHE[key_nc] = _build_nc(tpb2, epad, y2)
        nc_like = _CACHE[key_nc]
        _bir_cache_save(key_nc, nc_like)
    import threading
    _holder = {}

    def _bg_compile():
        try:
            _holder["c"] = _prepare_compiled(nc_like, n_groups)
        except Exception as e:  # surfaced at join
            _holder["e"] = e

    _th = threading.Thread(target=_bg_compile)
    _th.start()

    order = np.argsort(key, kind="stable")
    starts = np.concatenate([[0], np.cumsum(counts.reshape(-1))]).astype(np.int64)
    blk_off = np.concatenate(
        [[0], np.cumsum(np.asarray(tpb2) * 256)]).astype(np.int64)

    src_s = src[order]
    dst_s = dst[order]
    dl_s = dl[order].astype(np.float32)

    # packed per-core edge stream: (src, dst_global, dstloc_f32bits)
    ed3 = np.empty((NC_, epad, 3), np.int32)
    ed3[:, :, 0] = PADROW
    ed3[:, :, 1] = 0
    ed3[:, :, 2] = np.float32(0.0).view(np.int32)
    for c in range(NC_):
        for b in range(NBLK):
            k = c * NBLK + b
            s0, s1 = starts[k], starts[k + 1]
            cnt = s1 - s0
            o0 = blk_off[b]
            ed3[c, o0:o0 + cnt, 0] = src_s[s0:s1]
            ed3[c, o0:o0 + cnt, 1] = dst_s[s0:s1]
            ed3[c, o0:o0 + cnt, 2] = dl_s[s0:s1].view(np.int32)
    # regroup into [n_groups, 128, 6*IG]; per-iter column layout
    # [src_lo, src_hi, dstg_lo, dstg_hi, dlf_lo, dlf_hi]
    ed4 = np.zeros((NC_, n_groups * IG, P, 6), np.int32)
    ed4[:, :, :, 0:2] = PADROW
    ed4[:, :, :, 4:6] = np.float32(0.0).view(np.int32)
    e5 = ed3.reshape(NC_, n_iters, 2, P, 3)
    ed4[:, :n_iters, :, 0] = e5[:, :, 0, :, 0]
    ed4[:, :n_iters, :, 1] = e5[:, :, 1, :, 0]
    ed4[:, :n_iters, :, 2] = e5[:, :, 0, :, 1]
    ed4[:, :n_iters, :, 3] = e5[:, :, 1, :, 1]
    ed4[:, :n_iters, :, 4] = e5[:, :, 0, :, 2]
    ed4[:, :n_iters, :, 5] = e5[:, :, 1, :, 2]
    ed4 = np.ascontiguousarray(
        ed4.reshape(NC_, n_groups * IG, P, 6).transpose(0, 2, 1, 3)
        .reshape(NC_, P, n_groups * IG * 6))

    # ---- per-core final-permutation quarter-row indices ----
    p_ar = np.arange(P, dtype=np.int64)
    b_ar = np.arange(NBLK, dtype=np.int64)
    fidx_all = []
    for c in range(NC_):
        r = c * RPC + b_ar[None, :, None] * P + p_ar[:, None, None]
        j = np.arange(4, dtype=np.int64)[None, None, :]
        f = 4 * r + j
        h = f // N
        n = f - h * N
        rp = (n // RPC) * RPAD + (n % RPC)
        q = rp * 4 + h
        q = np.where(r < N, q, 0)
        fidx_all.append(q.reshape(P, NBLK * 4).astype(np.int32))

    xpad = np.zeros((NPAD, DIN), np.float16)
    for c in range(NC_):
        xpad[c * RPAD:c * RPAD + RPC] = x[c * RPC:(c + 1) * RPC].astype(np.float16)
    t0 = _tick("host prep", t0)

    ai_g = np.empty((NC_ * P, DIN), np.float32)
    aj_g = np.empty((NC_ * P, DIN), np.float32)
    for c in range(NC_):
        hsel = c // 2
        ai_g[c * P:(c + 1) * P] = np.concatenate([att[hsel, :DH]] * 4)[None, :]
        aj_g[c * P:(c + 1) * P] = np.concatenate([att[hsel, DH:]] * 4)[None, :]
    globals_in = [
        xpad,
        np.tile(wrhs, (NC_, 1)),
        np.tile(hb_b, (NC_, 1)),
        ai_g,
        aj_g,
        np.tile(bc_b, (NC_, 1)),
        ed4.reshape(NC_ * P, -1),
        np.concatenate(fidx_all, axis=0),
    ]

    def _mk_in_maps():
        return [
            {n: g[c * (g.shape[0] // NC_):(c + 1) * (g.shape[0] // NC_)]
             for n, g in zip(_IN_NAMES, globals_in)}
            for c in range(NC_)
        ]

    try:
        _th.join()
        if "e" in _holder:
            raise _holder["e"]
        t0 = _tick("compile join", t0)
        flat = _run_spmd(_holder["c"], globals_in)
        t0 = _tick("spmd run", t0)
        out = np.empty((N, DIN), np.float32)
        for c in range(NC_):
            out[c * RPC:(c + 1) * RPC] = flat[c * RPAD:c * RPAD + RPC]
    except Exception:
        import traceback
        traceback.print_exc()
        if key_nc not in _CACHE:
            _CACHE.clear()
            _CACHE[key_nc] = _build_nc(tpb2, epad, y2)
        r = run_bass_kernel_spmd(_CACHE[key_nc], _mk_in_maps(),
                                 list(range(NC_)), trace=False)
        t0 = _tick("spmd run (fallback)", t0)
        out = np.concatenate(
            [r.results[c]["y_out"][:RPC] for c in range(NC_)], axis=0)
        out = out.astype(np.float32)
    _tick("gather out", t0)
    return out


# ---------------- host fallback (no scipy, slow but correct) --------------
def _host_kernel(x, W, b_lin, att, b_conv, ei):
    x = np.asarray(x, dtype=np.float32)
    xh = _proj(_expmap0(_logmap0(x) @ np.asarray(W, np.float32).T))
    hb = _proj(_expmap0(np.asarray(b_lin, np.float32)[None, :]))
    x2 = (xh * xh).sum(-1, keepdims=True)
    b2 = (hb * hb).sum(-1, keepdims=True)
    xy = (xh * hb).sum(-1, keepdims=True)
    numer = (1 + 2 * xy + b2) * xh + (1 - x2) * hb
    denom = np.clip(1 + 2 * xy + x2 * b2, MIN_NORM, None)
    xh = _proj(numer / denom)
    L = _logmap0(xh)
    Lf = L.reshape(-1)
    G = np.empty((N, H * DH), np.float32)
    for h in range(H):
        G[:, h * DH:(h + 1) * DH] = Lf[h * N * DH:(h + 1) * N * DH].reshape(N, DH)
    si = (G.reshape(N, H, DH) * att[None, :, :DH]).sum(-1).astype(np.float32)
    sj = (G.reshape(N, H, DH) * att[None, :, DH:]).sum(-1).astype(np.float32)
    loop = np.arange(N, dtype=np.int64)
    srcv = np.concatenate([ei[0], loop])
    dstv = np.concatenate([ei[1], loop])
    alpha = si[dstv] + sj[srcv]
    alpha = np.where(alpha > 0, alpha, np.float32(0.2) * alpha)
    w = np.exp(alpha)
    den = np.zeros((N, H), np.float32)
    for h in range(H):
        den[:, h] = np.bincount(dstv, weights=w[:, h], minlength=N)
    order = np.argsort(dstv, kind="stable")
    ds = dstv[order]
    seg = np.concatenate([[0], np.flatnonzero(np.diff(ds)) + 1])
    seg_ids = ds[seg]
    msg = G[srcv[order]].reshape(-1, H, DH) * w[order][:, :, None]
    sums = np.add.reduceat(msg.reshape(-1, H * DH), seg, axis=0)
    numg = np.zeros((N, H * DH), np.float32)
    numg[seg_ids] = sums
    outg = numg.reshape(N, H, DH) / den[:, :, None]
    final = outg.transpose(1, 0, 2).reshape(N, H * DH)
    final = final + np.asarray(b_conv, np.float32)
    final = np.maximum(final, 0.0)
    return _proj(_expmap0(final)).astype(np.float32)


def kernel(x, edge_index, W, b_lin, att, b_conv):
    ei = np.asarray(edge_index).astype(np.int64)
    try:
        return _device_kernel(x, W, b_lin, att, b_conv, ei)
    except Exception:
        import traceback
        traceback.print_exc()
        return _host_kernel(np.asarray(x), np.asarray(W), np.asarray(b_lin),
                            np.asarray(att), np.asarray(b_conv), ei)
